# revision 2
# baseline (speedup 1.0000x reference)
"""DFIRE score kernel for Trainium2 (8 NeuronCores, SPMD). Self-contained.

Strategy: atoms are host-sorted by type; the all-pairs upper triangle is
processed as 128-row x 128-col blocks over (sorted rows) x (type-padded,
ghost-filled columns).  Per block: the PE computes squared distances
(|xi|^2 + |xj|^2 - 2 xi.xj) and poisons self/triangle-duplicate and padded
pairs via an identity matmul against a host-prepared mask tile; the ACT
engine takes sqrt to the scaled distance ds = d/0.7; a fused custom DVE op
applies the sequence-separation and 19.6A-cutoff mask (dead pairs -> FLT_MAX);
a second fused custom DVE op evaluates the 28-bin linear-interpolation
hat basis against per-(row-atom, column-type) LUT rows and accumulates the
energy.  Pair blocks are sharded round-robin across the 8 cores; the host
sums the per-core, per-partition partials.
"""
import os
if "axon" not in os.environ.get("JAX_PLATFORMS", "axon"):
    # the device run needs the axon PJRT backend; keep cpu available too
    os.environ["JAX_PLATFORMS"] = "axon,cpu"
else:
    os.environ.setdefault("JAX_PLATFORMS", "axon,cpu")

from contextlib import ExitStack
import numpy as np

import concourse.bass as bass
import concourse.tile as tile
from concourse import bacc, mybir
from concourse._compat import with_exitstack
from concourse import bass_utils

F32 = mybir.dt.float32
N = 8192
NT = 85
NB = 28
SEG = 32
NSEG = 4
BLK = 128
BIG = 1.0e6
GHOST0 = 1.0e6
GHOSTSTEP = 1.0e3
N_CORES = 8

# --------------------------------------------------------------------------- #
# custom DVE ops (registered at import)
# --------------------------------------------------------------------------- #

from concourse.dve_ops import (
    DveOp, OPS, CUSTOM_DVE_SPECS, _SUB_OPCODE_FOR_NAME, has_src1,
)
from concourse.dve_spec import (
    C0, C1, C2, AluOp, MaxNeg, PageIdx, Spec, Src0, Src1, Zero, One,
    lower as _dve_lower, minn, relu, select, sq,
)
from concourse.dve_uop import DveOpSpec


def _mk_op(name, spec, subdim):
    if name in _SUB_OPCODE_FOR_NAME:
        for op in OPS:
            if op.name == name:
                return op
        raise RuntimeError(name)
    row = max(_SUB_OPCODE_FOR_NAME.values()) + 1
    assert row < 0x20
    shas = {}
    for ver in ("v3", "v4"):
        uops = _dve_lower(spec, ver=ver)
        shas[ver] = DveOpSpec(
            name=name, opcode=row, uops=uops, rd1_en=has_src1(spec)
        ).sha(ver)
    op = DveOp(name, spec, subdim=subdim, uops_sha=shas)
    OPS.append(op)
    CUSTOM_DVE_SPECS[name] = spec
    _SUB_OPCODE_FOR_NAME[name] = row
    return op


def _ref_masked_clip(in0, in1, s0, s1, imm2):
    ds = in0.astype(np.float32)
    rj = in1.astype(np.float32)
    keep = ((rj - s0) ** 2 > s1) & (ds < imm2)
    big = np.float32(np.finfo(np.float32).max)
    return np.where(keep, np.minimum(ds, np.float32(imm2) - np.float32(1.0)),
                    big).astype(np.float32)


MASKED_CLIP_DFIRE = _mk_op(
    "MASKED_CLIP_DFIRE",
    Spec(
        body=select((sq(Src1 - C0) > C1) & (Src0 < C2),
                    minn(Src0, C2 - One), Zero - MaxNeg),
        reference=_ref_masked_clip,
    ),
    subdim=False,
)


def _ref_hat_lut(in0, in1, s0, s1, imm2):
    x = in0.astype(np.float32)
    P, S, _ = x.shape
    v = np.arange(S, dtype=np.float32)[None, :, None]
    p = x - (v - np.float32(1.0))
    hat = np.maximum(np.minimum(p, np.float32(2.0) - p), 0.0).astype(np.float32)
    body = (hat * in1.astype(np.float32)).astype(np.float32)
    acc = body.reshape(P, -1).sum(axis=-1, keepdims=True)
    return body, acc.astype(np.float32)


_hl_p = Src0 - PageIdx(Zero - One, One)
HAT_LUT_REDUCE_DFIRE = _mk_op(
    "HAT_LUT_REDUCE_DFIRE",
    Spec(
        body=relu(minn(_hl_p, (One + One) - _hl_p)) * Src1,
        accum=AluOp.ADD,
        reference=_ref_hat_lut,
    ),
    subdim=True,
)

# --------------------------------------------------------------------------- #
# host-side data preparation
# --------------------------------------------------------------------------- #


def prepare(coords, pot, res_ids, type_indices):
    coords = np.asarray(coords, np.float32)
    pot = np.asarray(pot, np.float32)
    res = np.asarray(res_ids, np.int64)
    typ = np.asarray(type_indices, np.int64)

    perm = np.argsort(typ, kind="stable")
    s_coords = coords[perm]
    s_res = res[perm].astype(np.float32)
    s_typ = typ[perm]

    col_atom = []
    col_typ = []
    for t in range(NT):
        idx = np.nonzero(s_typ == t)[0]
        col_atom.extend(idx.tolist())
        col_typ.extend([t] * len(idx))
        pad = (-len(idx)) % SEG
        col_atom.extend([-1] * pad)
        col_typ.extend([t] * pad)
    while len(col_atom) % BLK != 0:
        col_atom.extend([-1] * SEG)
        col_typ.extend([0] * SEG)
    col_atom = np.array(col_atom, np.int64)
    col_typ = np.array(col_typ, np.int64)
    NC = len(col_atom)
    NBJ = NC // BLK

    ghost_pos = GHOST0 + GHOSTSTEP * np.arange((col_atom < 0).sum(),
                                               dtype=np.float32)
    c_coords = np.zeros((NC, 3), np.float32)
    realm = col_atom >= 0
    c_coords[realm] = s_coords[col_atom[realm]]
    c_coords[~realm, 0] = ghost_pos
    c_coords[~realm, 1] = ghost_pos
    c_coords[~realm, 2] = ghost_pos
    c_res = np.zeros(NC, np.float32)
    c_res[realm] = s_res[col_atom[realm]]
    c_rank = np.where(realm, col_atom, -1)

    row_nsq = (s_coords * s_coords).sum(1)
    col_nsq = (c_coords.astype(np.float64) ** 2).sum(1).astype(np.float32)

    NBI = N // BLK
    units = []
    blk_maxrank = c_rank.reshape(NBJ, BLK).max(1)
    bigint = np.iinfo(np.int64).max
    blk_minrank = np.where(realm, c_rank, bigint).reshape(NBJ, BLK).min(1)
    for bi in range(NBI):
        for bj in range(NBJ):
            if blk_maxrank[bj] > bi * BLK:
                units.append((bi, bj))
    T = len(units)
    U = -(-T // N_CORES)

    smalls = np.zeros((N_CORES, U, 8, 3 * BLK), np.float32)
    triri = np.zeros((N_CORES, U, BLK, BLK + 1), np.float32)
    Rarr = np.zeros((N_CORES, U, BLK, NSEG * NB), np.float32)

    rowrank = np.arange(BLK)
    for k, (bi, bj) in enumerate(units):
        c = k % N_CORES
        u = k // N_CORES
        r0, r1 = bi * BLK, (bi + 1) * BLK
        j0, j1 = bj * BLK, (bj + 1) * BLK
        sm = smalls[c, u]
        sm[0:3, 0:BLK] = s_coords[r0:r1].T
        sm[3, 0:BLK] = row_nsq[r0:r1]
        sm[4, 0:BLK] = 1.0
        sm[0:3, BLK:2 * BLK] = -2.0 * c_coords[j0:j1].T
        sm[3, BLK:2 * BLK] = 1.0
        sm[4, BLK:2 * BLK] = col_nsq[j0:j1]
        sm[0, 2 * BLK:3 * BLK] = c_res[j0:j1]
        triri[c, u, :, BLK] = s_res[r0:r1]
        if blk_minrank[bj] <= bi * BLK + BLK - 1:
            tri = (c_rank[j0:j1][None, :] <= (r0 + rowrank)[:, None])
            triri[c, u, :, 0:BLK] = tri.astype(np.float32)
        t1b = s_typ[r0:r1]
        t2b = col_typ[j0 + SEG * np.arange(NSEG)]
        Rarr[c, u] = pot[t1b[:, None], t2b[None, :], :].reshape(BLK, NSEG * NB)
    for k in range(T, U * N_CORES):
        c = k % N_CORES
        u = k // N_CORES
        triri[c, u, :, 0:BLK] = 1.0

    return {"U": U, "smalls": smalls, "triri": triri, "R": Rarr}


# --------------------------------------------------------------------------- #
# device program
# --------------------------------------------------------------------------- #

@with_exitstack
def dfire_program(ctx: ExitStack, tc: tile.TileContext, U: int,
                  smalls_d, triri_d, R_d, bigi_d, out_d):
    nc = tc.nc
    const = ctx.enter_context(tc.tile_pool(name="const", bufs=1))
    sm_pool = ctx.enter_context(tc.tile_pool(name="sm", bufs=3))
    tri_pool = ctx.enter_context(tc.tile_pool(name="tri", bufs=3))
    r_pool = ctx.enter_context(tc.tile_pool(name="r", bufs=3))
    work = ctx.enter_context(tc.tile_pool(name="work", bufs=3))
    scr_pool = ctx.enter_context(tc.tile_pool(name="scr", bufs=2))
    acc_pool = ctx.enter_context(tc.tile_pool(name="acc", bufs=1))
    ps = ctx.enter_context(tc.tile_pool(name="ps", bufs=2, space="PSUM"))

    bigi = const.tile([128, 128], F32)
    nc.sync.dma_start(bigi[:], bigi_d[:])
    ones = const.tile([1, 128], F32)
    nc.gpsimd.memset(ones[:], 1.0)
    slots = acc_pool.tile([128, NSEG * U], F32)

    for u in range(U):
        sm = sm_pool.tile([8, 3 * BLK], F32)
        nc.scalar.dma_start(sm[:], smalls_d[u])
        tri = tri_pool.tile([BLK, BLK + 1], F32)
        nc.sync.dma_start(tri[:], triri_d[u])
        R = r_pool.tile([BLK, NSEG * NB], F32)
        nc.gpsimd.dma_start(R[:], R_d[u])

        d2 = ps.tile([128, BLK], F32)
        nc.tensor.matmul(d2[:], sm[:, 0:BLK], sm[:, BLK:2 * BLK],
                         start=True, stop=False, skip_group_check=True)
        nc.tensor.matmul(d2[:], bigi[:], tri[:, 0:BLK], start=False, stop=True,
                         skip_group_check=True)

        resjb = ps.tile([128, BLK], F32)
        nc.tensor.matmul(resjb[:], ones[:], sm[0:1, 2 * BLK:3 * BLK],
                         start=True, stop=True, skip_group_check=True)

        ds = work.tile([128, BLK], F32)
        nc.scalar.activation(ds[:], d2[:], mybir.ActivationFunctionType.Sqrt,
                             scale=float(1.0 / 0.49))

        dsp = work.tile([128, BLK], F32)
        nc.vector._custom_dve(
            MASKED_CLIP_DFIRE,
            out=dsp[:], in0=ds[:], in1=resjb[:],
            s0=tri[:, BLK:BLK + 1], s1=7.0, imm2=28.0,
        )

        scratch = scr_pool.tile([128, NB, SEG], F32)
        for s in range(NSEG):
            ds_seg = dsp[:, s * SEG:(s + 1) * SEG].unsqueeze(1).to_broadcast(
                (128, NB, SEG))
            r_seg = R[:, s * NB:(s + 1) * NB].to_broadcast((128, NB, SEG))
            nc.vector._custom_dve(
                HAT_LUT_REDUCE_DFIRE,
                out=scratch[:], in0=ds_seg, in1=r_seg,
                accum_out=slots[:, NSEG * u + s:NSEG * u + s + 1],
            )

    out_t = work.tile([128, 1], F32)
    nc.vector.tensor_reduce(out_t[:], slots[:], mybir.AxisListType.X,
                            mybir.AluOpType.add)
    nc.sync.dma_start(out_d[:], out_t[:])


_CACHE = {}


def _build(U):
    if U in _CACHE:
        return _CACHE[U]
    nc = bacc.Bacc("TRN2", target_bir_lowering=False, debug=False,
                   num_devices=N_CORES)
    smalls_d = nc.dram_tensor("smalls", [U, 8, 3 * BLK], F32,
                              kind="ExternalInput").ap()
    triri_d = nc.dram_tensor("triri", [U, BLK, BLK + 1], F32,
                             kind="ExternalInput").ap()
    R_d = nc.dram_tensor("rlut", [U, BLK, NSEG * NB], F32,
                         kind="ExternalInput").ap()
    bigi_d = nc.dram_tensor("bigi", [128, 128], F32, kind="ExternalInput").ap()
    out_d = nc.dram_tensor("out", [128, 1], F32, kind="ExternalOutput").ap()
    with tile.TileContext(nc) as tc:
        dfire_program(tc, U, smalls_d, triri_d, R_d, bigi_d, out_d)
    nc.compile()
    _CACHE[U] = nc
    return nc


def run(coords, pot_tensor, res_ids, type_indices, trace=False):
    data = prepare(coords, pot_tensor, res_ids, type_indices)
    U = data["U"]
    nc = _build(U)
    bigi = (BIG * np.eye(128)).astype(np.float32)
    in_maps = [
        {"smalls": data["smalls"][c], "triri": data["triri"][c],
         "rlut": data["R"][c], "bigi": bigi}
        for c in range(N_CORES)
    ]
    res = bass_utils.run_bass_kernel_spmd(
        nc, in_maps, core_ids=list(range(N_CORES)), trace=trace,
    )
    total = np.float64(0.0)
    for c in range(N_CORES):
        total += np.asarray(res.results[c]["out"], np.float64).sum()
    return np.float32(total), res


# --------------------------------------------------------------------------- #
# public entry
# --------------------------------------------------------------------------- #

def _is_triu(i_idx, j_idx, n=N):
    if len(i_idx) != n * (n - 1) // 2 or len(j_idx) != len(i_idx):
        return False
    counts = n - 1 - np.arange(n - 1)
    expect_i = np.repeat(np.arange(n - 1, dtype=np.int64), counts)
    if not np.array_equal(np.asarray(i_idx, np.int64), expect_i):
        return False
    starts = np.zeros(n - 1, np.int64)
    starts[1:] = np.cumsum(counts)[:-1]
    expect_j = np.arange(len(j_idx), dtype=np.int64) \
        - np.repeat(starts, counts) + expect_i + 1
    return np.array_equal(np.asarray(j_idx, np.int64), expect_j)


def _fallback(coords, pot_tensor, res_ids, type_indices, i_idx, j_idx):
    total = np.float64(0.0)
    P = len(i_idx)
    step = 1 << 22
    for a in range(0, P, step):
        ii = i_idx[a:a + step].astype(np.int64)
        jj = j_idx[a:a + step].astype(np.int64)
        sep = np.abs(res_ids[ii].astype(np.int64) - res_ids[jj].astype(np.int64))
        mask = sep > 2
        d = np.sqrt(((coords[ii] - coords[jj]) ** 2).astype(np.float32).sum(1))
        d = (d + np.float32(1e-8)).astype(np.float32)
        dsc = d / np.float32(0.7)
        d0 = np.floor(np.clip(dsc, 0.0, 27.0)).astype(np.int64)
        d1 = np.minimum(d0 + 1, 27)
        alpha = (dsc - d0.astype(np.float32)).astype(np.float32)
        t1 = type_indices[ii].astype(np.int64)
        t2 = type_indices[jj].astype(np.int64)
        e0 = pot_tensor[t1, t2, d0]
        e1 = pot_tensor[t1, t2, d1]
        en = ((1.0 - alpha) * e0 + alpha * e1).astype(np.float32)
        w = mask & (d < np.float32(19.6))
        total += np.float64((en * w).sum(dtype=np.float32))
    return np.float32(total)


def kernel(coords, pot_tensor, res_ids, type_indices, i_idx, j_idx):
    coords = np.asarray(coords)
    pot_tensor = np.asarray(pot_tensor)
    res_ids = np.asarray(res_ids)
    type_indices = np.asarray(type_indices)
    i_idx = np.asarray(i_idx)
    j_idx = np.asarray(j_idx)
    if len(coords) != N or not _is_triu(i_idx, j_idx, len(coords)):
        return _fallback(coords, pot_tensor, res_ids, type_indices,
                         i_idx, j_idx)
    out, _ = kernel_run_cached(coords, pot_tensor, res_ids, type_indices)
    return out


def kernel_run_cached(coords, pot_tensor, res_ids, type_indices, trace=False):
    return run(coords, pot_tensor, res_ids, type_indices, trace=trace)


# revision 4
# speedup vs baseline: 1.0143x; 1.0143x over previous
"""DFIRE score kernel for Trainium2 (8 NeuronCores, SPMD). Self-contained.

Strategy: atoms are host-sorted by type; the all-pairs upper triangle is
processed as 128-row x 128-col blocks over (sorted rows) x (type-padded,
ghost-filled columns).  Per block: the PE computes squared distances
(|xi|^2 + |xj|^2 - 2 xi.xj) and poisons self/triangle-duplicate and padded
pairs via an identity matmul against a host-prepared mask tile; the ACT
engine takes sqrt to the scaled distance ds = d/0.7; a fused custom DVE op
applies the sequence-separation and 19.6A-cutoff mask (dead pairs -> FLT_MAX);
a second fused custom DVE op evaluates the 28-bin linear-interpolation
hat basis against per-(row-atom, column-type) LUT rows and accumulates the
energy.  Pair blocks are sharded round-robin across the 8 cores; the host
sums the per-core, per-partition partials.
"""
import os
if "axon" not in os.environ.get("JAX_PLATFORMS", "axon"):
    # the device run needs the axon PJRT backend; keep cpu available too
    os.environ["JAX_PLATFORMS"] = "axon,cpu"
else:
    os.environ.setdefault("JAX_PLATFORMS", "axon,cpu")

from contextlib import ExitStack
import numpy as np

import concourse.bass as bass
import concourse.tile as tile
from concourse import bacc, mybir
from concourse._compat import with_exitstack
from concourse import bass_utils

F32 = mybir.dt.float32
N = 8192
NT = 85
NB = 28
SEG = 32
NSEG = 4
BLK = 128
BIG = 1.0e6
GHOST0 = 1.0e6
GHOSTSTEP = 1.0e3
N_CORES = 8

# --------------------------------------------------------------------------- #
# custom DVE ops (registered at import)
# --------------------------------------------------------------------------- #

from concourse.dve_ops import (
    DveOp, OPS, CUSTOM_DVE_SPECS, _SUB_OPCODE_FOR_NAME, has_src1,
)
from concourse.dve_spec import (
    C0, C1, C2, AluOp, MaxNeg, PageIdx, Spec, Src0, Src1, Zero, One,
    lower as _dve_lower, minn, relu, select, sq,
)
from concourse.dve_uop import DveOpSpec


def _mk_op(name, spec, subdim):
    if name in _SUB_OPCODE_FOR_NAME:
        for op in OPS:
            if op.name == name:
                return op
        raise RuntimeError(name)
    row = max(_SUB_OPCODE_FOR_NAME.values()) + 1
    assert row < 0x20
    shas = {}
    for ver in ("v3", "v4"):
        uops = _dve_lower(spec, ver=ver)
        shas[ver] = DveOpSpec(
            name=name, opcode=row, uops=uops, rd1_en=has_src1(spec)
        ).sha(ver)
    op = DveOp(name, spec, subdim=subdim, uops_sha=shas)
    OPS.append(op)
    CUSTOM_DVE_SPECS[name] = spec
    _SUB_OPCODE_FOR_NAME[name] = row
    return op


def _ref_masked_clip(in0, in1, s0, s1, imm2):
    ds = in0.astype(np.float32)
    rj = in1.astype(np.float32)
    keep = ((rj - s0) ** 2 > s1) & (ds < imm2)
    big = np.float32(np.finfo(np.float32).max)
    return np.where(keep, np.minimum(ds, np.float32(imm2) - np.float32(1.0)),
                    big).astype(np.float32)


MASKED_CLIP_DFIRE = _mk_op(
    "MASKED_CLIP_DFIRE",
    Spec(
        body=select((sq(Src1 - C0) > C1) & (Src0 < C2),
                    minn(Src0, C2 - One), Zero - MaxNeg),
        reference=_ref_masked_clip,
    ),
    subdim=False,
)


def _ref_hat_lut(in0, in1, s0, s1, imm2):
    x = in0.astype(np.float32)
    P, S, _ = x.shape
    v = np.arange(S, dtype=np.float32)[None, :, None]
    p = x - (v - np.float32(1.0))
    hat = np.maximum(np.minimum(p, np.float32(2.0) - p), 0.0).astype(np.float32)
    body = (hat * in1.astype(np.float32)).astype(np.float32)
    acc = body.reshape(P, -1).sum(axis=-1, keepdims=True)
    return body, acc.astype(np.float32)


_hl_p = Src0 - PageIdx(Zero - One, One)
HAT_LUT_REDUCE_DFIRE = _mk_op(
    "HAT_LUT_REDUCE_DFIRE",
    Spec(
        body=relu(minn(_hl_p, (One + One) - _hl_p)) * Src1,
        accum=AluOp.ADD,
        reference=_ref_hat_lut,
    ),
    subdim=True,
)

# --------------------------------------------------------------------------- #
# host-side data preparation
# --------------------------------------------------------------------------- #


def prepare(coords, pot, res_ids, type_indices):
    coords = np.asarray(coords, np.float32)
    pot = np.asarray(pot, np.float32)
    res = np.asarray(res_ids, np.int64)
    typ = np.asarray(type_indices, np.int64)

    perm = np.argsort(typ, kind="stable")
    s_coords = coords[perm]
    s_res = res[perm].astype(np.float32)
    s_typ = typ[perm]

    col_atom = []
    col_typ = []
    for t in range(NT):
        idx = np.nonzero(s_typ == t)[0]
        col_atom.extend(idx.tolist())
        col_typ.extend([t] * len(idx))
        pad = (-len(idx)) % SEG
        col_atom.extend([-1] * pad)
        col_typ.extend([t] * pad)
    while len(col_atom) % BLK != 0:
        col_atom.extend([-1] * SEG)
        col_typ.extend([0] * SEG)
    col_atom = np.array(col_atom, np.int64)
    col_typ = np.array(col_typ, np.int64)
    NC = len(col_atom)
    NBJ = NC // BLK

    ghost_pos = GHOST0 + GHOSTSTEP * np.arange((col_atom < 0).sum(),
                                               dtype=np.float32)
    c_coords = np.zeros((NC, 3), np.float32)
    realm = col_atom >= 0
    c_coords[realm] = s_coords[col_atom[realm]]
    c_coords[~realm, 0] = ghost_pos
    c_coords[~realm, 1] = ghost_pos
    c_coords[~realm, 2] = ghost_pos
    c_res = np.zeros(NC, np.float32)
    c_res[realm] = s_res[col_atom[realm]]
    c_rank = np.where(realm, col_atom, -1)

    row_nsq = (s_coords * s_coords).sum(1)
    col_nsq = (c_coords.astype(np.float64) ** 2).sum(1).astype(np.float32)

    NBI = N // BLK
    units = []
    blk_maxrank = c_rank.reshape(NBJ, BLK).max(1)
    bigint = np.iinfo(np.int64).max
    blk_minrank = np.where(realm, c_rank, bigint).reshape(NBJ, BLK).min(1)
    for bi in range(NBI):
        for bj in range(NBJ):
            if blk_maxrank[bj] > bi * BLK:
                units.append((bi, bj))
    T = len(units)
    U = -(-T // N_CORES)

    smalls = np.zeros((N_CORES, U, 8, 3 * BLK), np.float32)
    triri = np.zeros((N_CORES, U, BLK, BLK + 1), np.float32)
    Rarr = np.zeros((N_CORES, U, BLK, NSEG * NB), np.float32)

    rowrank = np.arange(BLK)
    for k, (bi, bj) in enumerate(units):
        c = k % N_CORES
        u = k // N_CORES
        r0, r1 = bi * BLK, (bi + 1) * BLK
        j0, j1 = bj * BLK, (bj + 1) * BLK
        sm = smalls[c, u]
        sm[0:3, 0:BLK] = s_coords[r0:r1].T
        sm[3, 0:BLK] = row_nsq[r0:r1]
        sm[4, 0:BLK] = 1.0
        sm[0:3, BLK:2 * BLK] = -2.0 * c_coords[j0:j1].T
        sm[3, BLK:2 * BLK] = 1.0
        sm[4, BLK:2 * BLK] = col_nsq[j0:j1]
        sm[0, 2 * BLK:3 * BLK] = c_res[j0:j1]
        triri[c, u, :, BLK] = s_res[r0:r1]
        if blk_minrank[bj] <= bi * BLK + BLK - 1:
            tri = (c_rank[j0:j1][None, :] <= (r0 + rowrank)[:, None])
            triri[c, u, :, 0:BLK] = tri.astype(np.float32)
        t1b = s_typ[r0:r1]
        t2b = col_typ[j0 + SEG * np.arange(NSEG)]
        Rarr[c, u] = pot[t1b[:, None], t2b[None, :], :].reshape(BLK, NSEG * NB)
    for k in range(T, U * N_CORES):
        c = k % N_CORES
        u = k // N_CORES
        triri[c, u, :, 0:BLK] = 1.0

    return {"U": U, "smalls": smalls, "triri": triri, "R": Rarr}


# --------------------------------------------------------------------------- #
# device program
# --------------------------------------------------------------------------- #

@with_exitstack
def dfire_program(ctx: ExitStack, tc: tile.TileContext, U: int,
                  smalls_d, triri_d, R_d, bigi_d, out_d):
    nc = tc.nc
    const = ctx.enter_context(tc.tile_pool(name="const", bufs=1))
    sm_pool = ctx.enter_context(tc.tile_pool(name="sm", bufs=6))
    tri_pool = ctx.enter_context(tc.tile_pool(name="tri", bufs=6))
    r_pool = ctx.enter_context(tc.tile_pool(name="r", bufs=6))
    work = ctx.enter_context(tc.tile_pool(name="work", bufs=6))
    scr_pool = ctx.enter_context(tc.tile_pool(name="scr", bufs=4))
    acc_pool = ctx.enter_context(tc.tile_pool(name="acc", bufs=1))
    ps = ctx.enter_context(tc.tile_pool(name="ps", bufs=3, space="PSUM"))

    bigi = const.tile([128, 128], F32)
    nc.sync.dma_start(bigi[:], bigi_d[:])
    ones = const.tile([1, 128], F32)
    nc.gpsimd.memset(ones[:], 1.0)
    slots = acc_pool.tile([128, NSEG * U], F32)

    for u in range(U):
        sm = sm_pool.tile([8, 3 * BLK], F32)
        nc.scalar.dma_start(sm[:], smalls_d[u])
        tri = tri_pool.tile([BLK, BLK + 1], F32)
        nc.sync.dma_start(tri[:], triri_d[u])
        R = r_pool.tile([BLK, NSEG * NB], F32)
        nc.gpsimd.dma_start(R[:], R_d[u])

        d2 = ps.tile([128, BLK], F32)
        nc.tensor.matmul(d2[:], sm[:, 0:BLK], sm[:, BLK:2 * BLK],
                         start=True, stop=False, skip_group_check=True)
        nc.tensor.matmul(d2[:], bigi[:], tri[:, 0:BLK], start=False, stop=True,
                         skip_group_check=True)

        resjb = work.tile([128, BLK], F32)
        nc.gpsimd.partition_broadcast(resjb[:], sm[0:1, 2 * BLK:3 * BLK])

        ds = work.tile([128, BLK], F32)
        nc.scalar.activation(ds[:], d2[:], mybir.ActivationFunctionType.Sqrt,
                             scale=float(1.0 / 0.49))

        dsp = work.tile([128, BLK], F32)
        nc.vector._custom_dve(
            MASKED_CLIP_DFIRE,
            out=dsp[:], in0=ds[:], in1=resjb[:],
            s0=tri[:, BLK:BLK + 1], s1=7.0, imm2=28.0,
        )

        scratch = scr_pool.tile([128, NB, SEG], F32)
        for s in range(NSEG):
            ds_seg = dsp[:, s * SEG:(s + 1) * SEG].unsqueeze(1).to_broadcast(
                (128, NB, SEG))
            r_seg = R[:, s * NB:(s + 1) * NB].to_broadcast((128, NB, SEG))
            nc.vector._custom_dve(
                HAT_LUT_REDUCE_DFIRE,
                out=scratch[:], in0=ds_seg, in1=r_seg,
                accum_out=slots[:, NSEG * u + s:NSEG * u + s + 1],
            )

    out_t = work.tile([128, 1], F32)
    nc.vector.tensor_reduce(out_t[:], slots[:], mybir.AxisListType.X,
                            mybir.AluOpType.add)
    nc.sync.dma_start(out_d[:], out_t[:])


_CACHE = {}


def _build(U):
    if U in _CACHE:
        return _CACHE[U]
    nc = bacc.Bacc("TRN2", target_bir_lowering=False, debug=False,
                   num_devices=N_CORES)
    smalls_d = nc.dram_tensor("smalls", [U, 8, 3 * BLK], F32,
                              kind="ExternalInput").ap()
    triri_d = nc.dram_tensor("triri", [U, BLK, BLK + 1], F32,
                             kind="ExternalInput").ap()
    R_d = nc.dram_tensor("rlut", [U, BLK, NSEG * NB], F32,
                         kind="ExternalInput").ap()
    bigi_d = nc.dram_tensor("bigi", [128, 128], F32, kind="ExternalInput").ap()
    out_d = nc.dram_tensor("out", [128, 1], F32, kind="ExternalOutput").ap()
    with tile.TileContext(nc) as tc:
        dfire_program(tc, U, smalls_d, triri_d, R_d, bigi_d, out_d)
    nc.compile()
    _CACHE[U] = nc
    return nc


def run(coords, pot_tensor, res_ids, type_indices, trace=False):
    data = prepare(coords, pot_tensor, res_ids, type_indices)
    U = data["U"]
    nc = _build(U)
    bigi = (BIG * np.eye(128)).astype(np.float32)
    in_maps = [
        {"smalls": data["smalls"][c], "triri": data["triri"][c],
         "rlut": data["R"][c], "bigi": bigi}
        for c in range(N_CORES)
    ]
    res = bass_utils.run_bass_kernel_spmd(
        nc, in_maps, core_ids=list(range(N_CORES)), trace=trace,
    )
    total = np.float64(0.0)
    for c in range(N_CORES):
        total += np.asarray(res.results[c]["out"], np.float64).sum()
    return np.float32(total), res


# --------------------------------------------------------------------------- #
# public entry
# --------------------------------------------------------------------------- #

def _is_triu(i_idx, j_idx, n=N):
    if len(i_idx) != n * (n - 1) // 2 or len(j_idx) != len(i_idx):
        return False
    counts = n - 1 - np.arange(n - 1)
    expect_i = np.repeat(np.arange(n - 1, dtype=np.int64), counts)
    if not np.array_equal(np.asarray(i_idx, np.int64), expect_i):
        return False
    starts = np.zeros(n - 1, np.int64)
    starts[1:] = np.cumsum(counts)[:-1]
    expect_j = np.arange(len(j_idx), dtype=np.int64) \
        - np.repeat(starts, counts) + expect_i + 1
    return np.array_equal(np.asarray(j_idx, np.int64), expect_j)


def _fallback(coords, pot_tensor, res_ids, type_indices, i_idx, j_idx):
    total = np.float64(0.0)
    P = len(i_idx)
    step = 1 << 22
    for a in range(0, P, step):
        ii = i_idx[a:a + step].astype(np.int64)
        jj = j_idx[a:a + step].astype(np.int64)
        sep = np.abs(res_ids[ii].astype(np.int64) - res_ids[jj].astype(np.int64))
        mask = sep > 2
        d = np.sqrt(((coords[ii] - coords[jj]) ** 2).astype(np.float32).sum(1))
        d = (d + np.float32(1e-8)).astype(np.float32)
        dsc = d / np.float32(0.7)
        d0 = np.floor(np.clip(dsc, 0.0, 27.0)).astype(np.int64)
        d1 = np.minimum(d0 + 1, 27)
        alpha = (dsc - d0.astype(np.float32)).astype(np.float32)
        t1 = type_indices[ii].astype(np.int64)
        t2 = type_indices[jj].astype(np.int64)
        e0 = pot_tensor[t1, t2, d0]
        e1 = pot_tensor[t1, t2, d1]
        en = ((1.0 - alpha) * e0 + alpha * e1).astype(np.float32)
        w = mask & (d < np.float32(19.6))
        total += np.float64((en * w).sum(dtype=np.float32))
    return np.float32(total)


def kernel(coords, pot_tensor, res_ids, type_indices, i_idx, j_idx):
    coords = np.asarray(coords)
    pot_tensor = np.asarray(pot_tensor)
    res_ids = np.asarray(res_ids)
    type_indices = np.asarray(type_indices)
    i_idx = np.asarray(i_idx)
    j_idx = np.asarray(j_idx)
    if len(coords) != N or not _is_triu(i_idx, j_idx, len(coords)):
        return _fallback(coords, pot_tensor, res_ids, type_indices,
                         i_idx, j_idx)
    out, _ = kernel_run_cached(coords, pot_tensor, res_ids, type_indices)
    return out


def kernel_run_cached(coords, pot_tensor, res_ids, type_indices, trace=False):
    return run(coords, pot_tensor, res_ids, type_indices, trace=trace)
